# revision 1
# baseline (speedup 1.0000x reference)
"""CapsuleLayer (dynamic routing, N_IN=512, N_OUT=2, D=16, 3 iters) on 8 trn2
NeuronCores, pure data-parallel over the batch.

v2: bf16 compute, BLK=256, no logit-carry buffer (iter-2 re-streams uz1 into
the same PSUM accumulator), tile_position packing for the block-diagonal
OnesRep matmul.

Per core (B_local=512, two blocks of 256 batch elems):
  - u is DMA'd (f32->bf16 cast in flight) then PE-transposed to
    uT [(ns,j)=128p x (g=64, b)]
  - P^T[ki,b] = sum_g Wbig[g]^T @ uT[g]  (PSUM-accumulated)
  - per routing iter (never materializes u_hat):
      z    = Z @ v^T              (Z[(ki),(nj)] = +W (k=0) / -W (k=1))
      uz_r = uT * z               (DVE bf16)
      dsum = Ones @ uz_1 [+ Ones @ uz_2]   (PE j-reduction, PSUM accumulate
                                            across iters; Ones is 32x32
                                            block-diagonal -> 4 packed tiles)
      sig  = sigmoid(dsum)        (ACT reads PSUM, writes bf16)
      utw  = sig * uT             (DVE bf16)
      t^T  = sum_g Wbig[g]^T @ utw[g]
      s^T  = t*kmask + P*pmask;  v = squash(s)   (f32 small ops)
  - v3^T [32 x b] DMA'd out; host transposes/reshapes.

softmax over k=2 == sigmoid of logit diff; squash(s) = g(|s|^2)*s.
"""

import numpy as np

N_CORES = 8
B = 4096
B_LOCAL = B // N_CORES          # 512
BLK = 256                       # batch block per pass
N_BLOCKS = B_LOCAL // BLK       # 2
N_IN, N_OUT, D = 512, 2, 16
G = 64                          # n-groups of 8 capsules: 8*16 = 128 partitions
KI = N_OUT * D                  # 32
EPS = 1e-07

_CACHE = {}


# ---------------------------------------------------------------------------
# Walrus in this container allows only ONE sync-wait per TPB instruction.
# Tile attaches several sem waits to one instruction; split extras onto
# standalone NoOps (same engine, one wait each) inserted just before it.
# ---------------------------------------------------------------------------
def _apply_tile_patch():
    import concourse.tile as tile_mod
    from concourse import mybir
    from concourse.vector_clock import ScopedClock
    from concourse._compat import nn

    if getattr(tile_mod.TileContext, "_wait_split_patched", False):
        return

    _orig_add_instruction = tile_mod.TileContext._add_instruction

    def _split_waits(self, inst):
        si = inst.sync_info
        if si is None or len(si.on_wait) <= 1:
            return
        waits = list(si.on_wait)
        ups = list(si.on_update)
        inst.sync_info = mybir.SyncInfo(on_wait=[waits[-1]], on_update=ups)
        for i, w in enumerate(waits[:-1]):
            nop = mybir.InstNoOp(name=f"{inst.name}-wsplit{i}", ins=[], outs=[])
            nop.engine = inst.engine
            nop.sync_info = mybir.SyncInfo(on_wait=[w], on_update=[])
            self.nc.register_instruction(nop, overwrite=True)
            nn(self.nc.cur_bb).bb.add_instruction(nop)

    def _patched_add_instruction(self, inst):
        _split_waits(self, inst)
        _orig_add_instruction(self, inst)

    def _patched_drain_and_barrier(self, tick_clock, wait_clock):
        nc = self.nc
        drain_inst = nc.sync.drain()
        wait_clock.add_sem_waits(
            drain_inst.ins, ScopedClock({None: tick_clock.global_clock})
        )
        si = drain_inst.ins.sync_info
        if si is not None and len(si.on_wait) > 1:
            waits = list(si.on_wait)
            ups = list(si.on_update)
            drain_inst.ins.sync_info = mybir.SyncInfo(
                on_wait=[waits[0]], on_update=ups
            )
            for w in waits[1:]:
                nop = nc.sync.nop(nofuse=True)
                nop.ins.sync_info = mybir.SyncInfo(on_wait=[w], on_update=[])

        nc.all_engine_barrier()
        assert self.sems is not None
        popped = nc._tile_sem_poison_stack.pop()
        assert popped is self._sem_poison
        nc.clear_and_free_semaphores(list(self.sems.allocated().values()))
        nc.all_engine_barrier()

    tile_mod.TileContext._add_instruction = _patched_add_instruction
    tile_mod.TileContext._drain_and_barrier = _patched_drain_and_barrier
    tile_mod.TileContext._wait_split_patched = True


# ---------------------------------------------------------------------------
# Host-side constant prep from W  (W: [1, 512, 2, 16, 16] f32, idx [_,n,k,i,j])
# ---------------------------------------------------------------------------
def _prep_consts(W):
    import ml_dtypes

    bf16 = ml_dtypes.bfloat16
    W = np.asarray(W, dtype=np.float32).reshape(N_IN, N_OUT, D, D)  # [n,k,i,j]
    # Wbig[g][(ns*16+j), (k*16+i)] = W[8g+ns, k, i, j]
    Wg = W.reshape(G, 8, N_OUT, D, D)                   # [g, ns, k, i, j]
    wbig = np.transpose(Wg, (0, 1, 4, 2, 3)).reshape(G, 128, KI).copy()
    # Z[g][(k*16+i), (ns*16+j)] = sign(k) * W[8g+ns, k, i, j]
    z = np.transpose(Wg, (0, 2, 3, 1, 4)).reshape(G, KI, 128).copy()
    z[:, D:, :] *= -1.0                                  # k=1 rows negative
    # OnesRep[(ns,j), (ns',j')] = 1 iff ns==ns'  (32x32 block-diagonal);
    # store the 4 diagonal 32x32 blocks stacked as [128, 32]
    ones_rep = np.kron(np.eye(8, dtype=np.float32), np.ones((16, 16), np.float32))
    ones_diag = np.stack(
        [ones_rep[32 * a : 32 * a + 32, 32 * a : 32 * a + 32] for a in range(4)]
    ).reshape(128, KI)  # kept for possible packed variant
    # OnesK[(k,i), (k',i')] = 1 iff k==k'
    ones_k = np.kron(np.eye(2, dtype=np.float32), np.ones((16, 16), np.float32))
    ident = np.eye(128, dtype=np.float32)
    km = np.concatenate([np.ones(D, np.float32), -np.ones(D, np.float32)])[:, None]
    pmsk = np.concatenate([np.zeros(D, np.float32), np.ones(D, np.float32)])[:, None]
    return {
        "wbig": np.ascontiguousarray(wbig.astype(bf16)),           # [64, 128, 32]
        "zmat": np.ascontiguousarray(z.astype(bf16)),              # [64, 32, 128]
        "onesdiag": np.ascontiguousarray(ones_rep.astype(bf16)),   # [128, 128]
        "onesk": np.ascontiguousarray(ones_k.astype(bf16)),        # [32, 32]
        "ident": np.ascontiguousarray(ident.astype(bf16)),         # [128, 128]
        "kmask": np.ascontiguousarray(km),                         # [32, 1] f32
        "pmask": np.ascontiguousarray(pmsk),                       # [32, 1] f32
    }


# ---------------------------------------------------------------------------
# Bass program
# ---------------------------------------------------------------------------
def _build_program(repeat=1, cast_via_dma=True):
    import concourse.bass as bass
    import concourse.tile as tile
    from concourse import mybir

    _apply_tile_patch()
    f32 = mybir.dt.float32
    bf16 = mybir.dt.bfloat16

    nc = bass.Bass(trn_type="TRN2", target_bir_lowering=False)
    u_in = nc.declare_dram_parameter("u", [B_LOCAL, N_IN * D], f32, isOutput=False)
    wbig_in = nc.declare_dram_parameter("wbig", [G, 128, KI], bf16, isOutput=False)
    z_in = nc.declare_dram_parameter("zmat", [G, KI, 128], bf16, isOutput=False)
    onesdiag_in = nc.declare_dram_parameter(
        "onesdiag", [128, 128], bf16, isOutput=False
    )
    onesk_in = nc.declare_dram_parameter("onesk", [KI, KI], bf16, isOutput=False)
    ident_in = nc.declare_dram_parameter("ident", [128, 128], bf16, isOutput=False)
    kmask_in = nc.declare_dram_parameter("kmask", [KI, 1], f32, isOutput=False)
    pmask_in = nc.declare_dram_parameter("pmask", [KI, 1], f32, isOutput=False)
    v_out = nc.declare_dram_parameter("v", [KI, B_LOCAL], f32, isOutput=True)

    Sig = mybir.ActivationFunctionType.Sigmoid
    Sqrt = mybir.ActivationFunctionType.Sqrt

    with tile.TileContext(nc) as tc:
        import contextlib

        with contextlib.ExitStack() as ctx:
            consts = ctx.enter_context(tc.tile_pool(name="consts", bufs=1))
            stage_p = ctx.enter_context(tc.tile_pool(name="stage", bufs=2))
            ut_p = ctx.enter_context(tc.tile_pool(name="ut", bufs=1))
            bufa_p = ctx.enter_context(tc.tile_pool(name="bufa", bufs=1))
            bufb_p = ctx.enter_context(tc.tile_pool(name="bufb", bufs=1))
            bufc_p = ctx.enter_context(tc.tile_pool(name="bufc", bufs=1))
            small_p = ctx.enter_context(tc.tile_pool(name="small", bufs=2))
            vsm_p = ctx.enter_context(tc.tile_pool(name="vsm", bufs=2))
            ps_t = ctx.enter_context(tc.tile_pool(name="ps_t", bufs=2, space="PSUM"))
            ps_z = ctx.enter_context(tc.tile_pool(name="ps_z", bufs=2, space="PSUM"))
            ps_d = ctx.enter_context(tc.tile_pool(name="ps_d", bufs=2, space="PSUM"))
            ps_s = ctx.enter_context(tc.tile_pool(name="ps_s", bufs=2, space="PSUM"))

            # --- constants to SBUF
            wbig = consts.tile([128, G, KI], bf16)        # lhsT slices [:,g,:]
            for g in range(G):
                nc.sync.dma_start(out=wbig[:, g, :], in_=wbig_in[g, :, :])
            zmat = consts.tile([KI, G, 128], bf16)
            for g in range(G):
                nc.sync.dma_start(out=zmat[:, g, :], in_=z_in[g, :, :])
            onesdiag = consts.tile([128, 128], bf16)
            nc.sync.dma_start(out=onesdiag, in_=onesdiag_in[:, :])
            onesk = consts.tile([KI, KI], bf16)
            nc.sync.dma_start(out=onesk, in_=onesk_in[:, :])
            ident = consts.tile([128, 128], bf16)
            nc.sync.dma_start(out=ident, in_=ident_in[:, :])
            kmask = consts.tile([KI, 1], f32)
            nc.sync.dma_start(out=kmask, in_=kmask_in[:, :])
            pmask = consts.tile([KI, 1], f32)
            nc.sync.dma_start(out=pmask, in_=pmask_in[:, :])

            import contextlib as _ctxlib

            rep_cm = tc.For_i(0, repeat, 1) if repeat > 1 else _ctxlib.nullcontext()

            def squash(s_sb, tag):
                """s_sb [KI, BLK] f32 -> v^T bf16 [KI, BLK]."""
                s2 = small_p.tile([KI, BLK], bf16, tag="s2")
                nc.scalar.square(s2, s_sb)
                nsq_ps = ps_s.tile([KI, BLK], f32, tag="kps")
                nc.tensor.matmul(nsq_ps, onesk, s2, start=True, stop=True)
                sqr = small_p.tile([KI, BLK], f32, tag="sqr")
                nc.scalar.activation(sqr, nsq_ps, Sqrt)
                nc.vector.tensor_scalar_add(sqr, sqr, EPS)
                nsq1 = small_p.tile([KI, BLK], f32, tag="nsq1")
                nc.vector.tensor_scalar_add(nsq1, nsq_ps, 1.0)
                nc.vector.tensor_mul(nsq1, nsq1, sqr)
                nc.vector.reciprocal(nsq1, nsq1)
                gfac = small_p.tile([KI, BLK], f32, tag="gfac")
                nc.vector.tensor_mul(gfac, nsq_ps, nsq1)
                vt = vsm_p.tile([KI, BLK], f32, tag="vt" + tag)
                nc.vector.tensor_mul(vt, s_sb, gfac)
                vtb = vsm_p.tile([KI, BLK], bf16, tag="vtb" + tag)
                nc.vector.tensor_copy(out=vtb, in_=vt)
                return vt, vtb

            def matphase(out_ps, rhs_full):
                """sum_g wbig[g]^T @ rhs_full[:,g,:] into out_ps [KI, BLK]."""
                for g in range(G):
                    nc.tensor.matmul(
                        out_ps, wbig[:, g, :], rhs_full[:, g, :],
                        start=(g == 0), stop=(g == G - 1),
                    )

            with rep_cm:
              for blk in range(N_BLOCKS):
                b0 = blk * BLK
                # --- load + transpose: uT [(ns,j)=128, (g, b)] bf16
                ut = ut_p.tile([128, G, BLK], bf16, tag="ut")
                for sb in range(BLK // 128):
                    for ck in range(4):
                        u_slice = u_in[
                            b0 + 128 * sb : b0 + 128 * (sb + 1),
                            2048 * ck : 2048 * (ck + 1),
                        ]
                        stage = stage_p.tile([128, 16 * 128], bf16, tag="stage")
                        if cast_via_dma:
                            nc.gpsimd.dma_start(out=stage, in_=u_slice)
                        else:
                            stage_f = stage_p.tile(
                                [128, 16 * 128], f32, tag="stagef"
                            )
                            nc.sync.dma_start(out=stage_f, in_=u_slice)
                            nc.vector.tensor_copy(out=stage, in_=stage_f)
                        for q in range(4):
                            tp = ps_t.tile([128, 4, 128], bf16, tag="tp")
                            for r in range(4):
                                gl = 4 * q + r
                                nc.tensor.transpose(
                                    tp[:, r, :],
                                    stage[:, 128 * gl : 128 * (gl + 1)],
                                    ident,
                                )
                            g0 = 16 * ck + 4 * q
                            nc.vector.tensor_copy(
                                out=ut[:, g0 : g0 + 4, 128 * sb : 128 * (sb + 1)],
                                in_=tp,
                            )

                # --- P^T = sum_g Wbig[g]^T @ uT[g]
                p_ps = ps_s.tile([KI, BLK], f32, tag="kps")
                matphase(p_ps, ut)
                p_sb = small_p.tile([KI, BLK], f32, tag="psb")
                nc.vector.tensor_copy(out=p_sb, in_=p_ps)
                pmp = small_p.tile([KI, BLK], f32, tag="pmp")
                nc.vector.tensor_scalar_mul(pmp, p_sb, pmask)

                # --- iter 1: c = 0.5 -> s1 = 0.5 * P
                s1 = small_p.tile([KI, BLK], f32, tag="s1")
                nc.vector.tensor_scalar_mul(s1, p_sb, 0.5)
                _, vtb = squash(s1, "i0")

                uz1 = bufb_p.tile([128, G, BLK], bf16, tag="bufb")

                for it in range(2):
                    # z = Z @ v^T   (buffer A for it0, C for it1)
                    zpool, ztag = (bufa_p, "bufa") if it == 0 else (bufc_p, "bufc")
                    za = zpool.tile([128, G, BLK], bf16, tag=ztag)
                    for h in range(G // 2):
                        zp = ps_z.tile([128, 2, BLK], f32, tag="zp")
                        for r in range(2):
                            g = 2 * h + r
                            nc.tensor.matmul(
                                zp[:, r, :], zmat[:, g, :], vtb,
                                start=True, stop=True,
                            )
                        sl = za[:, 2 * h : 2 * h + 2, :]
                        if h % 3 == 0:
                            nc.vector.tensor_copy(out=sl, in_=zp)
                        else:
                            nc.scalar.copy(out=sl, in_=zp)
                    # uz = uT * z
                    if it == 0:
                        uz = uz1                      # persists into iter 2
                    else:
                        uz = bufa_p.tile([128, G, BLK], bf16, tag="bufa")
                    for c8 in range(8):
                        gs = slice(8 * c8, 8 * c8 + 8)
                        nc.vector.tensor_mul(
                            uz[:, gs, :], ut[:, gs, :], za[:, gs, :]
                        )

                    # dsum[g] = Ones @ uz1[g] (+ Ones @ uz2[g] in iter 2);
                    # Ones is 32x32 block-diagonal -> 4 packed tile MMs
                    sig = bufc_p.tile([128, G, BLK], bf16, tag="bufc")
                    for h in range(G // 2):
                        dp = ps_d.tile([128, 2, BLK], f32, tag="dp")
                        for r in range(2):
                            g = 2 * h + r
                            srcs = [uz] if it == 0 else [uz1, uz]
                            for si_, src in enumerate(srcs):
                                nc.tensor.matmul(
                                    dp[:, r, :],
                                    onesdiag,
                                    src[:, g, :],
                                    start=(si_ == 0),
                                    stop=(si_ == len(srcs) - 1),
                                )
                        # sigma = sigmoid(dsum) straight out of PSUM
                        nc.scalar.activation(sig[:, 2 * h : 2 * h + 2, :], dp, Sig)
                    # utw = sig * uT
                    utw = bufa_p.tile([128, G, BLK], bf16, tag="bufa")
                    for c8 in range(8):
                        gs = slice(8 * c8, 8 * c8 + 8)
                        nc.vector.tensor_mul(
                            utw[:, gs, :], ut[:, gs, :], sig[:, gs, :]
                        )

                    t_ps = ps_s.tile([KI, BLK], f32, tag="kps")
                    matphase(t_ps, utw)
                    # s = t*kmask + P*pmask
                    s_sb = small_p.tile([KI, BLK], f32, tag="ssb")
                    nc.vector.scalar_tensor_tensor(
                        out=s_sb, in0=t_ps, scalar=kmask, in1=pmp,
                        op0=mybir.AluOpType.mult, op1=mybir.AluOpType.add,
                    )
                    vt_f32, vtb = squash(s_sb, f"i{it + 1}")

                nc.sync.dma_start(out=v_out[:, b0 : b0 + BLK], in_=vt_f32)

    return nc


def _get_program(repeat=1, cast_via_dma=True):
    key = ("nc", repeat, cast_via_dma)
    if key not in _CACHE:
        _CACHE[key] = _build_program(repeat, cast_via_dma)
    return _CACHE[key]


# ---------------------------------------------------------------------------
# Public entry: full inputs -> full output
# ---------------------------------------------------------------------------
def kernel(inputs, W):
    from concourse.bass_utils import run_bass_kernel_spmd

    inputs = np.asarray(inputs, dtype=np.float32)
    u_full = inputs.reshape(B, N_IN * D)
    consts = _prep_consts(W)

    nc = _get_program()
    in_maps = []
    for c in range(N_CORES):
        m = {"u": np.ascontiguousarray(u_full[c * B_LOCAL : (c + 1) * B_LOCAL])}
        m.update(consts)
        in_maps.append(m)

    res = run_bass_kernel_spmd(nc, in_maps, list(range(N_CORES)))
    outs = []
    for c in range(N_CORES):
        vt = res.results[c]["v"]                  # [KI, B_LOCAL]
        outs.append(vt.T)                         # [B_LOCAL, KI]
    v = np.concatenate(outs, axis=0)              # [B, 32]
    return np.ascontiguousarray(v.reshape(B, 1, N_OUT, D).astype(np.float32))



# revision 3
# speedup vs baseline: 1.0842x; 1.0842x over previous
"""CapsuleLayer (dynamic routing, N_IN=512, N_OUT=2, D=16, 3 iters) on 8 trn2
NeuronCores, pure data-parallel over the batch.

v2: bf16 compute, BLK=256, no logit-carry buffer (iter-2 re-streams uz1 into
the same PSUM accumulator), tile_position packing for the block-diagonal
OnesRep matmul.

Per core (B_local=512, two blocks of 256 batch elems):
  - u is DMA'd (f32->bf16 cast in flight) then PE-transposed to
    uT [(ns,j)=128p x (g=64, b)]
  - P^T[ki,b] = sum_g Wbig[g]^T @ uT[g]  (PSUM-accumulated)
  - per routing iter (never materializes u_hat):
      z    = Z @ v^T              (Z[(ki),(nj)] = +W (k=0) / -W (k=1))
      uz_r = uT * z               (DVE bf16)
      dsum = Ones @ uz_1 [+ Ones @ uz_2]   (PE j-reduction, PSUM accumulate
                                            across iters; Ones is 32x32
                                            block-diagonal -> 4 packed tiles)
      sig  = sigmoid(dsum)        (ACT reads PSUM, writes bf16)
      utw  = sig * uT             (DVE bf16)
      t^T  = sum_g Wbig[g]^T @ utw[g]
      s^T  = t*kmask + P*pmask;  v = squash(s)   (f32 small ops)
  - v3^T [32 x b] DMA'd out; host transposes/reshapes.

softmax over k=2 == sigmoid of logit diff; squash(s) = g(|s|^2)*s.
"""

import numpy as np

N_CORES = 8
B = 4096
B_LOCAL = B // N_CORES          # 512
BLK = 256                       # batch block per pass
N_BLOCKS = B_LOCAL // BLK       # 2
N_IN, N_OUT, D = 512, 2, 16
G = 64                          # n-groups of 8 capsules: 8*16 = 128 partitions
KI = N_OUT * D                  # 32
EPS = 1e-07

_CACHE = {}


# ---------------------------------------------------------------------------
# Walrus in this container allows only ONE sync-wait per TPB instruction.
# Tile attaches several sem waits to one instruction; split extras onto
# standalone NoOps (same engine, one wait each) inserted just before it.
# ---------------------------------------------------------------------------
def _apply_tile_patch():
    import concourse.tile as tile_mod
    from concourse import mybir
    from concourse.vector_clock import ScopedClock
    from concourse._compat import nn

    if getattr(tile_mod.TileContext, "_wait_split_patched", False):
        return

    _orig_add_instruction = tile_mod.TileContext._add_instruction

    def _split_waits(self, inst):
        si = inst.sync_info
        if si is None or len(si.on_wait) <= 1:
            return
        waits = list(si.on_wait)
        ups = list(si.on_update)
        inst.sync_info = mybir.SyncInfo(on_wait=[waits[-1]], on_update=ups)
        for i, w in enumerate(waits[:-1]):
            nop = mybir.InstNoOp(name=f"{inst.name}-wsplit{i}", ins=[], outs=[])
            nop.engine = inst.engine
            nop.sync_info = mybir.SyncInfo(on_wait=[w], on_update=[])
            self.nc.register_instruction(nop, overwrite=True)
            nn(self.nc.cur_bb).bb.add_instruction(nop)

    def _patched_add_instruction(self, inst):
        _split_waits(self, inst)
        _orig_add_instruction(self, inst)

    def _patched_drain_and_barrier(self, tick_clock, wait_clock):
        nc = self.nc
        drain_inst = nc.sync.drain()
        wait_clock.add_sem_waits(
            drain_inst.ins, ScopedClock({None: tick_clock.global_clock})
        )
        si = drain_inst.ins.sync_info
        if si is not None and len(si.on_wait) > 1:
            waits = list(si.on_wait)
            ups = list(si.on_update)
            drain_inst.ins.sync_info = mybir.SyncInfo(
                on_wait=[waits[0]], on_update=ups
            )
            for w in waits[1:]:
                nop = nc.sync.nop(nofuse=True)
                nop.ins.sync_info = mybir.SyncInfo(on_wait=[w], on_update=[])

        nc.all_engine_barrier()
        assert self.sems is not None
        popped = nc._tile_sem_poison_stack.pop()
        assert popped is self._sem_poison
        nc.clear_and_free_semaphores(list(self.sems.allocated().values()))
        nc.all_engine_barrier()

    tile_mod.TileContext._add_instruction = _patched_add_instruction
    tile_mod.TileContext._drain_and_barrier = _patched_drain_and_barrier
    tile_mod.TileContext._wait_split_patched = True


# ---------------------------------------------------------------------------
# Host-side constant prep from W  (W: [1, 512, 2, 16, 16] f32, idx [_,n,k,i,j])
# ---------------------------------------------------------------------------
def _prep_consts(W):
    import ml_dtypes

    bf16 = ml_dtypes.bfloat16
    W = np.asarray(W, dtype=np.float32).reshape(N_IN, N_OUT, D, D)  # [n,k,i,j]
    # Wbig[g][(ns*16+j), (k*16+i)] = W[8g+ns, k, i, j]
    Wg = W.reshape(G, 8, N_OUT, D, D)                   # [g, ns, k, i, j]
    wbig = np.transpose(Wg, (0, 1, 4, 2, 3)).reshape(G, 128, KI).copy()
    # Z[g][(k*16+i), (ns*16+j)] = sign(k) * W[8g+ns, k, i, j]
    z = np.transpose(Wg, (0, 2, 3, 1, 4)).reshape(G, KI, 128).copy()
    z[:, D:, :] *= -1.0                                  # k=1 rows negative
    # OnesRep[(ns,j), (ns',j')] = 1 iff ns==ns'  (32x32 block-diagonal);
    # store the 4 diagonal 32x32 blocks stacked as [128, 32]
    ones_rep = np.kron(np.eye(8, dtype=np.float32), np.ones((16, 16), np.float32))
    ones_diag = np.stack(
        [ones_rep[32 * a : 32 * a + 32, 32 * a : 32 * a + 32] for a in range(4)]
    ).reshape(128, KI)  # kept for possible packed variant
    # OnesK[(k,i), (k',i')] = 1 iff k==k'
    ones_k = np.kron(np.eye(2, dtype=np.float32), np.ones((16, 16), np.float32))
    ident = np.eye(128, dtype=np.float32)
    km = np.concatenate([np.ones(D, np.float32), -np.ones(D, np.float32)])[:, None]
    pmsk = np.concatenate([np.zeros(D, np.float32), np.ones(D, np.float32)])[:, None]
    return {
        "wbig": np.ascontiguousarray(wbig.astype(bf16)),           # [64, 128, 32]
        "zmat": np.ascontiguousarray(z.astype(bf16)),              # [64, 32, 128]
        "onesdiag": np.ascontiguousarray(ones_rep.astype(bf16)),   # [128, 128]
        "onesk": np.ascontiguousarray(ones_k.astype(bf16)),        # [32, 32]
        "ident": np.ascontiguousarray(ident.astype(bf16)),         # [128, 128]
        "kmask": np.ascontiguousarray(km),                         # [32, 1] f32
        "pmask": np.ascontiguousarray(pmsk),                       # [32, 1] f32
    }


# ---------------------------------------------------------------------------
# Bass program
# ---------------------------------------------------------------------------
def _build_program(repeat=1, cast_via_dma=False):
    import concourse.bass as bass
    import concourse.tile as tile
    from concourse import mybir

    _apply_tile_patch()
    f32 = mybir.dt.float32
    bf16 = mybir.dt.bfloat16

    nc = bass.Bass(trn_type="TRN2", target_bir_lowering=False)
    u_in = nc.declare_dram_parameter("u", [B_LOCAL, N_IN * D], f32, isOutput=False)
    wbig_in = nc.declare_dram_parameter("wbig", [G, 128, KI], bf16, isOutput=False)
    z_in = nc.declare_dram_parameter("zmat", [G, KI, 128], bf16, isOutput=False)
    onesdiag_in = nc.declare_dram_parameter(
        "onesdiag", [128, 128], bf16, isOutput=False
    )
    onesk_in = nc.declare_dram_parameter("onesk", [KI, KI], bf16, isOutput=False)
    ident_in = nc.declare_dram_parameter("ident", [128, 128], bf16, isOutput=False)
    kmask_in = nc.declare_dram_parameter("kmask", [KI, 1], f32, isOutput=False)
    pmask_in = nc.declare_dram_parameter("pmask", [KI, 1], f32, isOutput=False)
    v_out = nc.declare_dram_parameter("v", [KI, B_LOCAL], f32, isOutput=True)

    Sig = mybir.ActivationFunctionType.Sigmoid
    Sqrt = mybir.ActivationFunctionType.Sqrt

    with tile.TileContext(nc) as tc:
        import contextlib

        with contextlib.ExitStack() as ctx:
            consts = ctx.enter_context(tc.tile_pool(name="consts", bufs=1))
            stage_p = ctx.enter_context(tc.tile_pool(name="stage", bufs=2))
            ut_p = ctx.enter_context(tc.tile_pool(name="ut", bufs=1))
            bufa_p = ctx.enter_context(tc.tile_pool(name="bufa", bufs=1))
            bufb_p = ctx.enter_context(tc.tile_pool(name="bufb", bufs=1))
            bufc_p = ctx.enter_context(tc.tile_pool(name="bufc", bufs=1))
            small_p = ctx.enter_context(tc.tile_pool(name="small", bufs=2))
            vsm_p = ctx.enter_context(tc.tile_pool(name="vsm", bufs=2))
            ps_t = ctx.enter_context(tc.tile_pool(name="ps_t", bufs=2, space="PSUM"))
            ps_z = ctx.enter_context(tc.tile_pool(name="ps_z", bufs=2, space="PSUM"))
            ps_d = ctx.enter_context(tc.tile_pool(name="ps_d", bufs=2, space="PSUM"))
            ps_s = ctx.enter_context(tc.tile_pool(name="ps_s", bufs=2, space="PSUM"))

            # --- constants to SBUF
            wbig = consts.tile([128, G, KI], bf16)        # lhsT slices [:,g,:]
            for g in range(G):
                nc.sync.dma_start(out=wbig[:, g, :], in_=wbig_in[g, :, :])
            zmat = consts.tile([KI, G, 128], bf16)
            for g in range(G):
                nc.sync.dma_start(out=zmat[:, g, :], in_=z_in[g, :, :])
            onesdiag = consts.tile([128, 128], bf16)
            nc.sync.dma_start(out=onesdiag, in_=onesdiag_in[:, :])
            onesk = consts.tile([KI, KI], bf16)
            nc.sync.dma_start(out=onesk, in_=onesk_in[:, :])
            ident = consts.tile([128, 128], bf16)
            nc.sync.dma_start(out=ident, in_=ident_in[:, :])
            kmask = consts.tile([KI, 1], f32)
            nc.sync.dma_start(out=kmask, in_=kmask_in[:, :])
            pmask = consts.tile([KI, 1], f32)
            nc.sync.dma_start(out=pmask, in_=pmask_in[:, :])

            import contextlib as _ctxlib

            rep_cm = tc.For_i(0, repeat, 1) if repeat > 1 else _ctxlib.nullcontext()

            def squash(s_sb, tag):
                """s_sb [KI, BLK] f32 -> v^T bf16 [KI, BLK]."""
                s2 = small_p.tile([KI, BLK], bf16, tag="s2")
                nc.scalar.square(s2, s_sb)
                nsq_ps = ps_s.tile([KI, BLK], f32, tag="kps")
                nc.tensor.matmul(nsq_ps, onesk, s2, start=True, stop=True)
                sqr = small_p.tile([KI, BLK], f32, tag="sqr")
                nc.scalar.activation(sqr, nsq_ps, Sqrt)
                nc.vector.tensor_scalar_add(sqr, sqr, EPS)
                nsq1 = small_p.tile([KI, BLK], f32, tag="nsq1")
                nc.vector.tensor_scalar_add(nsq1, nsq_ps, 1.0)
                nc.vector.tensor_mul(nsq1, nsq1, sqr)
                nc.vector.reciprocal(nsq1, nsq1)
                gfac = small_p.tile([KI, BLK], f32, tag="gfac")
                nc.vector.tensor_mul(gfac, nsq_ps, nsq1)
                vt = vsm_p.tile([KI, BLK], f32, tag="vt" + tag)
                nc.vector.tensor_mul(vt, s_sb, gfac)
                vtb = vsm_p.tile([KI, BLK], bf16, tag="vtb" + tag)
                nc.vector.tensor_copy(out=vtb, in_=vt)
                return vt, vtb

            def matphase(out_ps, rhs_full):
                """sum_g wbig[g]^T @ rhs_full[:,g,:] into out_ps [KI, BLK]."""
                for g in range(G):
                    nc.tensor.matmul(
                        out_ps, wbig[:, g, :], rhs_full[:, g, :],
                        start=(g == 0), stop=(g == G - 1),
                    )

            with rep_cm:
              for blk in range(N_BLOCKS):
                b0 = blk * BLK
                # --- load + transpose: uT [(ns,j)=128, (g, b)] bf16
                ut = ut_p.tile([128, G, BLK], bf16, tag="ut")
                for sb in range(BLK // 128):
                    for ck in range(4):
                        u_slice = u_in[
                            b0 + 128 * sb : b0 + 128 * (sb + 1),
                            2048 * ck : 2048 * (ck + 1),
                        ]
                        stage = stage_p.tile([128, 16 * 128], bf16, tag="stage")
                        if cast_via_dma:
                            nc.gpsimd.dma_start(out=stage, in_=u_slice)
                        else:
                            stage_f = stage_p.tile(
                                [128, 16 * 128], f32, tag="stagef"
                            )
                            nc.sync.dma_start(out=stage_f, in_=u_slice)
                            nc.vector.tensor_copy(out=stage, in_=stage_f)
                        for q in range(4):
                            tp = ps_t.tile([128, 4, 128], bf16, tag="tp")
                            for r in range(4):
                                gl = 4 * q + r
                                nc.tensor.transpose(
                                    tp[:, r, :],
                                    stage[:, 128 * gl : 128 * (gl + 1)],
                                    ident,
                                )
                            g0 = 16 * ck + 4 * q
                            nc.vector.tensor_copy(
                                out=ut[:, g0 : g0 + 4, 128 * sb : 128 * (sb + 1)],
                                in_=tp,
                            )

                # --- P^T = sum_g Wbig[g]^T @ uT[g]
                p_ps = ps_s.tile([KI, BLK], f32, tag="kps")
                matphase(p_ps, ut)
                p_sb = small_p.tile([KI, BLK], f32, tag="psb")
                nc.vector.tensor_copy(out=p_sb, in_=p_ps)
                pmp = small_p.tile([KI, BLK], f32, tag="pmp")
                nc.vector.tensor_scalar_mul(pmp, p_sb, pmask)

                # --- iter 1: c = 0.5 -> s1 = 0.5 * P
                s1 = small_p.tile([KI, BLK], f32, tag="s1")
                nc.vector.tensor_scalar_mul(s1, p_sb, 0.5)
                _, vtb = squash(s1, "i0")

                uz1 = bufb_p.tile([128, G, BLK], bf16, tag="bufb")

                for it in range(2):
                    # z = Z @ v^T   (buffer A for it0, C for it1)
                    zpool, ztag = (bufa_p, "bufa") if it == 0 else (bufc_p, "bufc")
                    za = zpool.tile([128, G, BLK], bf16, tag=ztag)
                    for h in range(G // 2):
                        zp = ps_z.tile([128, 2, BLK], f32, tag="zp")
                        for r in range(2):
                            g = 2 * h + r
                            nc.tensor.matmul(
                                zp[:, r, :], zmat[:, g, :], vtb,
                                start=True, stop=True,
                            )
                        sl = za[:, 2 * h : 2 * h + 2, :]
                        if h % 3 == 0:
                            nc.vector.tensor_copy(out=sl, in_=zp)
                        else:
                            nc.scalar.copy(out=sl, in_=zp)
                    # uz = uT * z
                    if it == 0:
                        uz = uz1                      # persists into iter 2
                    else:
                        uz = bufa_p.tile([128, G, BLK], bf16, tag="bufa")
                    for c8 in range(8):
                        gs = slice(8 * c8, 8 * c8 + 8)
                        nc.vector.tensor_mul(
                            uz[:, gs, :], ut[:, gs, :], za[:, gs, :]
                        )

                    # dsum[g] = Ones @ uz1[g] (+ Ones @ uz2[g] in iter 2);
                    # Ones is 32x32 block-diagonal -> 4 packed tile MMs
                    sig = bufc_p.tile([128, G, BLK], bf16, tag="bufc")
                    for h in range(G // 2):
                        dp = ps_d.tile([128, 2, BLK], f32, tag="dp")
                        for r in range(2):
                            g = 2 * h + r
                            srcs = [uz] if it == 0 else [uz1, uz]
                            for si_, src in enumerate(srcs):
                                nc.tensor.matmul(
                                    dp[:, r, :],
                                    onesdiag,
                                    src[:, g, :],
                                    start=(si_ == 0),
                                    stop=(si_ == len(srcs) - 1),
                                )
                        # sigma = sigmoid(dsum) straight out of PSUM
                        nc.scalar.activation(sig[:, 2 * h : 2 * h + 2, :], dp, Sig)
                    # utw = sig * uT
                    utw = bufa_p.tile([128, G, BLK], bf16, tag="bufa")
                    for c8 in range(8):
                        gs = slice(8 * c8, 8 * c8 + 8)
                        nc.vector.tensor_mul(
                            utw[:, gs, :], ut[:, gs, :], sig[:, gs, :]
                        )

                    t_ps = ps_s.tile([KI, BLK], f32, tag="kps")
                    matphase(t_ps, utw)
                    # s = t*kmask + P*pmask
                    s_sb = small_p.tile([KI, BLK], f32, tag="ssb")
                    nc.vector.scalar_tensor_tensor(
                        out=s_sb, in0=t_ps, scalar=kmask, in1=pmp,
                        op0=mybir.AluOpType.mult, op1=mybir.AluOpType.add,
                    )
                    vt_f32, vtb = squash(s_sb, f"i{it + 1}")

                nc.sync.dma_start(out=v_out[:, b0 : b0 + BLK], in_=vt_f32)

    return nc


def _get_program(repeat=1, cast_via_dma=False):
    key = ("nc", repeat, cast_via_dma)
    if key not in _CACHE:
        _CACHE[key] = _build_program(repeat, cast_via_dma)
    return _CACHE[key]


# ---------------------------------------------------------------------------
# Public entry: full inputs -> full output
# ---------------------------------------------------------------------------
def kernel(inputs, W):
    from concourse.bass_utils import run_bass_kernel_spmd

    inputs = np.asarray(inputs, dtype=np.float32)
    u_full = inputs.reshape(B, N_IN * D)
    consts = _prep_consts(W)

    nc = _get_program()
    in_maps = []
    for c in range(N_CORES):
        m = {"u": np.ascontiguousarray(u_full[c * B_LOCAL : (c + 1) * B_LOCAL])}
        m.update(consts)
        in_maps.append(m)

    res = run_bass_kernel_spmd(nc, in_maps, list(range(N_CORES)))
    outs = []
    for c in range(N_CORES):
        vt = res.results[c]["v"]                  # [KI, B_LOCAL]
        outs.append(vt.T)                         # [B_LOCAL, KI]
    v = np.concatenate(outs, axis=0)              # [B, 32]
    return np.ascontiguousarray(v.reshape(B, 1, N_OUT, D).astype(np.float32))



# revision 5
# speedup vs baseline: 1.7263x; 1.5922x over previous
"""CapsuleLayer (dynamic routing, N_IN=512, N_OUT=2, D=16, 3 iters) on 8 trn2
NeuronCores, pure data-parallel over the batch.

v2: bf16 compute, BLK=256, no logit-carry buffer (iter-2 re-streams uz1 into
the same PSUM accumulator), tile_position packing for the block-diagonal
OnesRep matmul.

Per core (B_local=512, two blocks of 256 batch elems):
  - u is DMA'd (f32->bf16 cast in flight) then PE-transposed to
    uT [(ns,j)=128p x (g=64, b)]
  - P^T[ki,b] = sum_g Wbig[g]^T @ uT[g]  (PSUM-accumulated)
  - per routing iter (never materializes u_hat):
      z    = Z @ v^T              (Z[(ki),(nj)] = +W (k=0) / -W (k=1))
      uz_r = uT * z               (DVE bf16)
      dsum = Ones @ uz_1 [+ Ones @ uz_2]   (PE j-reduction, PSUM accumulate
                                            across iters; Ones is 32x32
                                            block-diagonal -> 4 packed tiles)
      sig  = sigmoid(dsum)        (ACT reads PSUM, writes bf16)
      utw  = sig * uT             (DVE bf16)
      t^T  = sum_g Wbig[g]^T @ utw[g]
      s^T  = t*kmask + P*pmask;  v = squash(s)   (f32 small ops)
  - v3^T [32 x b] DMA'd out; host transposes/reshapes.

softmax over k=2 == sigmoid of logit diff; squash(s) = g(|s|^2)*s.
"""

import numpy as np

N_CORES = 8
B = 4096
B_LOCAL = B // N_CORES          # 512
BLK = 256                       # batch block per pass
N_BLOCKS = B_LOCAL // BLK       # 2
N_IN, N_OUT, D = 512, 2, 16
G = 64                          # n-groups of 8 capsules: 8*16 = 128 partitions
KI = N_OUT * D                  # 32
EPS = 1e-07

_CACHE = {}


# ---------------------------------------------------------------------------
# Walrus in this container allows only ONE sync-wait per TPB instruction.
# Tile attaches several sem waits to one instruction; split extras onto
# standalone NoOps (same engine, one wait each) inserted just before it.
# ---------------------------------------------------------------------------
def _apply_tile_patch():
    import concourse.tile as tile_mod
    from concourse import mybir
    from concourse.vector_clock import ScopedClock
    from concourse._compat import nn

    if getattr(tile_mod.TileContext, "_wait_split_patched", False):
        return

    _orig_add_instruction = tile_mod.TileContext._add_instruction

    def _split_waits(self, inst):
        si = inst.sync_info
        if si is None or len(si.on_wait) <= 1:
            return
        waits = list(si.on_wait)
        ups = list(si.on_update)
        inst.sync_info = mybir.SyncInfo(on_wait=[waits[-1]], on_update=ups)
        for i, w in enumerate(waits[:-1]):
            nop = mybir.InstNoOp(name=f"{inst.name}-wsplit{i}", ins=[], outs=[])
            nop.engine = inst.engine
            nop.sync_info = mybir.SyncInfo(on_wait=[w], on_update=[])
            self.nc.register_instruction(nop, overwrite=True)
            nn(self.nc.cur_bb).bb.add_instruction(nop)

    def _patched_add_instruction(self, inst):
        _split_waits(self, inst)
        _orig_add_instruction(self, inst)

    def _patched_drain_and_barrier(self, tick_clock, wait_clock):
        nc = self.nc
        drain_inst = nc.sync.drain()
        wait_clock.add_sem_waits(
            drain_inst.ins, ScopedClock({None: tick_clock.global_clock})
        )
        si = drain_inst.ins.sync_info
        if si is not None and len(si.on_wait) > 1:
            waits = list(si.on_wait)
            ups = list(si.on_update)
            drain_inst.ins.sync_info = mybir.SyncInfo(
                on_wait=[waits[0]], on_update=ups
            )
            for w in waits[1:]:
                nop = nc.sync.nop(nofuse=True)
                nop.ins.sync_info = mybir.SyncInfo(on_wait=[w], on_update=[])

        nc.all_engine_barrier()
        assert self.sems is not None
        popped = nc._tile_sem_poison_stack.pop()
        assert popped is self._sem_poison
        nc.clear_and_free_semaphores(list(self.sems.allocated().values()))
        nc.all_engine_barrier()

    tile_mod.TileContext._add_instruction = _patched_add_instruction
    tile_mod.TileContext._drain_and_barrier = _patched_drain_and_barrier
    tile_mod.TileContext._wait_split_patched = True


# ---------------------------------------------------------------------------
# Host-side constant prep from W  (W: [1, 512, 2, 16, 16] f32, idx [_,n,k,i,j])
# ---------------------------------------------------------------------------
def _prep_consts(W):
    import ml_dtypes

    bf16 = ml_dtypes.bfloat16
    W = np.asarray(W, dtype=np.float32).reshape(N_IN, N_OUT, D, D)  # [n,k,i,j]
    # Wbig[g][(ns*16+j), (k*16+i)] = W[8g+ns, k, i, j]
    Wg = W.reshape(G, 8, N_OUT, D, D)                   # [g, ns, k, i, j]
    wbig = np.transpose(Wg, (0, 1, 4, 2, 3)).reshape(G, 128, KI).copy()
    # Z[g][(k*16+i), (ns*16+j)] = sign(k) * W[8g+ns, k, i, j]
    z = np.transpose(Wg, (0, 2, 3, 1, 4)).reshape(G, KI, 128).copy()
    z[:, D:, :] *= -1.0                                  # k=1 rows negative
    # OnesRep[(ns,j), (ns',j')] = 1 iff ns==ns'  (32x32 block-diagonal);
    # store the 4 diagonal 32x32 blocks stacked as [128, 32]
    ones_rep = np.kron(np.eye(8, dtype=np.float32), np.ones((16, 16), np.float32))
    ones_diag = np.stack(
        [ones_rep[32 * a : 32 * a + 32, 32 * a : 32 * a + 32] for a in range(4)]
    ).reshape(128, KI)  # kept for possible packed variant
    # OnesK[(k,i), (k',i')] = 1 iff k==k'
    ones_k = np.kron(np.eye(2, dtype=np.float32), np.ones((16, 16), np.float32))
    ident = np.eye(128, dtype=np.float32)
    km = np.concatenate([np.ones(D, np.float32), -np.ones(D, np.float32)])[:, None]
    pmsk = np.concatenate([np.zeros(D, np.float32), np.ones(D, np.float32)])[:, None]
    return {
        "wbig": np.ascontiguousarray(
            np.transpose(wbig, (1, 0, 2)).reshape(128, G * KI).astype(bf16)
        ),                                                         # [128, 64*32]
        "zmat": np.ascontiguousarray(
            np.transpose(z, (1, 0, 2)).reshape(KI, G * 128).astype(bf16)
        ),                                                         # [32, 64*128]
        "onesdiag": np.ascontiguousarray(ones_rep.astype(bf16)),   # [128, 128]
        "onesk": np.ascontiguousarray(ones_k.astype(bf16)),        # [32, 32]
        "ident": np.ascontiguousarray(ident.astype(bf16)),         # [128, 128]
        "kmask": np.ascontiguousarray(km),                         # [32, 1] f32
        "pmask": np.ascontiguousarray(pmsk),                       # [32, 1] f32
    }


# ---------------------------------------------------------------------------
# Bass program
# ---------------------------------------------------------------------------
def _build_program(repeat=1, cast_via_dma=False):
    import concourse.bass as bass
    import concourse.tile as tile
    from concourse import mybir

    _apply_tile_patch()
    f32 = mybir.dt.float32
    bf16 = mybir.dt.bfloat16

    nc = bass.Bass(trn_type="TRN2", target_bir_lowering=False)
    u_in = nc.declare_dram_parameter("u", [B_LOCAL, N_IN * D], f32, isOutput=False)
    wbig_in = nc.declare_dram_parameter("wbig", [128, G * KI], bf16, isOutput=False)
    z_in = nc.declare_dram_parameter("zmat", [KI, G * 128], bf16, isOutput=False)
    onesdiag_in = nc.declare_dram_parameter(
        "onesdiag", [128, 128], bf16, isOutput=False
    )
    onesk_in = nc.declare_dram_parameter("onesk", [KI, KI], bf16, isOutput=False)
    ident_in = nc.declare_dram_parameter("ident", [128, 128], bf16, isOutput=False)
    kmask_in = nc.declare_dram_parameter("kmask", [KI, 1], f32, isOutput=False)
    pmask_in = nc.declare_dram_parameter("pmask", [KI, 1], f32, isOutput=False)
    v_out = nc.declare_dram_parameter("v", [KI, B_LOCAL], f32, isOutput=True)

    Sig = mybir.ActivationFunctionType.Sigmoid
    Sqrt = mybir.ActivationFunctionType.Sqrt

    with tile.TileContext(nc) as tc:
        import contextlib

        with contextlib.ExitStack() as ctx:
            consts = ctx.enter_context(tc.tile_pool(name="consts", bufs=1))
            stage_p = ctx.enter_context(tc.tile_pool(name="stage", bufs=2))
            ut_p = ctx.enter_context(tc.tile_pool(name="ut", bufs=1))
            bufa_p = ctx.enter_context(tc.tile_pool(name="bufa", bufs=1))
            bufb_p = ctx.enter_context(tc.tile_pool(name="bufb", bufs=1))
            bufc_p = ctx.enter_context(tc.tile_pool(name="bufc", bufs=1))
            small_p = ctx.enter_context(tc.tile_pool(name="small", bufs=2))
            vsm_p = ctx.enter_context(tc.tile_pool(name="vsm", bufs=2))
            ps_t = ctx.enter_context(tc.tile_pool(name="ps_t", bufs=2, space="PSUM"))
            ps_z = ctx.enter_context(tc.tile_pool(name="ps_z", bufs=2, space="PSUM"))
            ps_d = ctx.enter_context(tc.tile_pool(name="ps_d", bufs=2, space="PSUM"))
            ps_s = ctx.enter_context(tc.tile_pool(name="ps_s", bufs=2, space="PSUM"))

            # --- constants to SBUF
            wbig_t = consts.tile([128, G * KI], bf16)
            nc.sync.dma_start(out=wbig_t, in_=wbig_in[:, :])
            wbig = wbig_t.rearrange("p (g ki) -> p g ki", g=G)
            zmat_t = consts.tile([KI, G * 128], bf16)
            nc.sync.dma_start(out=zmat_t, in_=z_in[:, :])
            zmat = zmat_t.rearrange("p (g nj) -> p g nj", g=G)
            onesdiag = consts.tile([128, 128], bf16)
            nc.sync.dma_start(out=onesdiag, in_=onesdiag_in[:, :])
            onesk = consts.tile([KI, KI], bf16)
            nc.sync.dma_start(out=onesk, in_=onesk_in[:, :])
            ident = consts.tile([128, 128], bf16)
            nc.sync.dma_start(out=ident, in_=ident_in[:, :])
            kmask = consts.tile([KI, 1], f32)
            nc.sync.dma_start(out=kmask, in_=kmask_in[:, :])
            pmask = consts.tile([KI, 1], f32)
            nc.sync.dma_start(out=pmask, in_=pmask_in[:, :])

            import contextlib as _ctxlib

            rep_cm = tc.For_i(0, repeat, 1) if repeat > 1 else _ctxlib.nullcontext()

            def squash(s_sb, tag):
                """s_sb [KI, BLK] f32 -> v^T bf16 [KI, BLK]."""
                s2 = small_p.tile([KI, BLK], bf16, tag="s2")
                nc.scalar.square(s2, s_sb)
                nsq_ps = ps_s.tile([KI, BLK], f32, tag="kps")
                nc.tensor.matmul(nsq_ps, onesk, s2, start=True, stop=True)
                sqr = small_p.tile([KI, BLK], f32, tag="sqr")
                nc.scalar.activation(sqr, nsq_ps, Sqrt)
                nc.vector.tensor_scalar_add(sqr, sqr, EPS)
                nsq1 = small_p.tile([KI, BLK], f32, tag="nsq1")
                nc.vector.tensor_scalar_add(nsq1, nsq_ps, 1.0)
                nc.vector.tensor_mul(nsq1, nsq1, sqr)
                nc.vector.reciprocal(nsq1, nsq1)
                gfac = small_p.tile([KI, BLK], f32, tag="gfac")
                nc.vector.tensor_mul(gfac, nsq_ps, nsq1)
                vt = vsm_p.tile([KI, BLK], f32, tag="vt" + tag)
                nc.vector.tensor_mul(vt, s_sb, gfac)
                vtb = vsm_p.tile([KI, BLK], bf16, tag="vtb" + tag)
                nc.vector.tensor_copy(out=vtb, in_=vt)
                return vt, vtb

            def matphase(out_ps, rhs_full):
                """sum_g wbig[g]^T @ rhs_full[:,g,:] into out_ps [KI, BLK]."""
                for g in range(G):
                    nc.tensor.matmul(
                        out_ps, wbig[:, g, :], rhs_full[:, g, :],
                        start=(g == 0), stop=(g == G - 1),
                    )

            with rep_cm:
              for blk in range(N_BLOCKS):
                b0 = blk * BLK
                # --- load + transpose: uT [(ns,j)=128, (g, b)] bf16
                ut = ut_p.tile([128, G, BLK], bf16, tag="ut")
                for sb in range(BLK // 128):
                    for ck in range(4):
                        u_slice = u_in[
                            b0 + 128 * sb : b0 + 128 * (sb + 1),
                            2048 * ck : 2048 * (ck + 1),
                        ]
                        stage = stage_p.tile([128, 16 * 128], bf16, tag="stage")
                        if cast_via_dma:
                            nc.gpsimd.dma_start(out=stage, in_=u_slice)
                        else:
                            stage_f = stage_p.tile(
                                [128, 16 * 128], f32, tag="stagef"
                            )
                            nc.sync.dma_start(out=stage_f, in_=u_slice)
                            if ck % 2 == 0:
                                nc.vector.tensor_copy(out=stage, in_=stage_f)
                            else:
                                nc.scalar.copy(out=stage, in_=stage_f)
                        for q in range(4):
                            tp = ps_t.tile([128, 4, 128], bf16, tag="tp")
                            for r in range(4):
                                gl = 4 * q + r
                                nc.tensor.transpose(
                                    tp[:, r, :],
                                    stage[:, 128 * gl : 128 * (gl + 1)],
                                    ident,
                                )
                            g0 = 16 * ck + 4 * q
                            nc.vector.tensor_copy(
                                out=ut[:, g0 : g0 + 4, 128 * sb : 128 * (sb + 1)],
                                in_=tp,
                            )

                # --- P^T = sum_g Wbig[g]^T @ uT[g]
                p_ps = ps_s.tile([KI, BLK], f32, tag="kps")
                matphase(p_ps, ut)
                p_sb = small_p.tile([KI, BLK], f32, tag="psb")
                nc.vector.tensor_copy(out=p_sb, in_=p_ps)
                pmp = small_p.tile([KI, BLK], f32, tag="pmp")
                nc.vector.tensor_scalar_mul(pmp, p_sb, pmask)

                # --- iter 1: c = 0.5 -> s1 = 0.5 * P
                s1 = small_p.tile([KI, BLK], f32, tag="s1")
                nc.vector.tensor_scalar_mul(s1, p_sb, 0.5)
                _, vtb = squash(s1, "i0")

                uz1 = bufb_p.tile([128, G, BLK], bf16, tag="bufb")

                for it in range(2):
                    # z = Z @ v^T   (buffer A for it0, C for it1)
                    zpool, ztag = (bufa_p, "bufa") if it == 0 else (bufc_p, "bufc")
                    za = zpool.tile([128, G, BLK], bf16, tag=ztag)
                    for h in range(G // 2):
                        zp = ps_z.tile([128, 2, BLK], f32, tag="zp")
                        for r in range(2):
                            g = 2 * h + r
                            nc.tensor.matmul(
                                zp[:, r, :], zmat[:, g, :], vtb,
                                start=True, stop=True,
                            )
                        sl = za[:, 2 * h : 2 * h + 2, :]
                        if h % 3 == 0:
                            nc.vector.tensor_copy(out=sl, in_=zp)
                        else:
                            nc.scalar.copy(out=sl, in_=zp)
                    # uz = uT * z
                    if it == 0:
                        uz = uz1                      # persists into iter 2
                    else:
                        uz = bufa_p.tile([128, G, BLK], bf16, tag="bufa")
                    for c8 in range(8):
                        gs = slice(8 * c8, 8 * c8 + 8)
                        nc.vector.tensor_mul(
                            uz[:, gs, :], ut[:, gs, :], za[:, gs, :]
                        )

                    # dsum[g] = Ones @ uz1[g] (+ Ones @ uz2[g] in iter 2);
                    # Ones is 32x32 block-diagonal -> 4 packed tile MMs
                    sig = bufc_p.tile([128, G, BLK], bf16, tag="bufc")
                    for h in range(G // 2):
                        dp = ps_d.tile([128, 2, BLK], f32, tag="dp")
                        for r in range(2):
                            g = 2 * h + r
                            srcs = [uz] if it == 0 else [uz1, uz]
                            for si_, src in enumerate(srcs):
                                nc.tensor.matmul(
                                    dp[:, r, :],
                                    onesdiag,
                                    src[:, g, :],
                                    start=(si_ == 0),
                                    stop=(si_ == len(srcs) - 1),
                                )
                        # sigma = sigmoid(dsum) straight out of PSUM
                        nc.scalar.activation(sig[:, 2 * h : 2 * h + 2, :], dp, Sig)
                    # utw = sig * uT
                    utw = bufa_p.tile([128, G, BLK], bf16, tag="bufa")
                    for c8 in range(8):
                        gs = slice(8 * c8, 8 * c8 + 8)
                        nc.vector.tensor_mul(
                            utw[:, gs, :], ut[:, gs, :], sig[:, gs, :]
                        )

                    t_ps = ps_s.tile([KI, BLK], f32, tag="kps")
                    matphase(t_ps, utw)
                    # s = t*kmask + P*pmask
                    s_sb = small_p.tile([KI, BLK], f32, tag="ssb")
                    nc.vector.scalar_tensor_tensor(
                        out=s_sb, in0=t_ps, scalar=kmask, in1=pmp,
                        op0=mybir.AluOpType.mult, op1=mybir.AluOpType.add,
                    )
                    vt_f32, vtb = squash(s_sb, f"i{it + 1}")

                nc.sync.dma_start(out=v_out[:, b0 : b0 + BLK], in_=vt_f32)

    return nc


def _get_program(repeat=1, cast_via_dma=False):
    key = ("nc", repeat, cast_via_dma)
    if key not in _CACHE:
        _CACHE[key] = _build_program(repeat, cast_via_dma)
    return _CACHE[key]


# ---------------------------------------------------------------------------
# Public entry: full inputs -> full output
# ---------------------------------------------------------------------------
def kernel(inputs, W):
    from concourse.bass_utils import run_bass_kernel_spmd

    inputs = np.asarray(inputs, dtype=np.float32)
    u_full = inputs.reshape(B, N_IN * D)
    consts = _prep_consts(W)

    nc = _get_program()
    in_maps = []
    for c in range(N_CORES):
        m = {"u": np.ascontiguousarray(u_full[c * B_LOCAL : (c + 1) * B_LOCAL])}
        m.update(consts)
        in_maps.append(m)

    res = run_bass_kernel_spmd(nc, in_maps, list(range(N_CORES)))
    outs = []
    for c in range(N_CORES):
        vt = res.results[c]["v"]                  # [KI, B_LOCAL]
        outs.append(vt.T)                         # [B_LOCAL, KI]
    v = np.concatenate(outs, axis=0)              # [B, 32]
    return np.ascontiguousarray(v.reshape(B, 1, N_OUT, D).astype(np.float32))

